# revision 2
# baseline (speedup 1.0000x reference)
"""Batched GAT (dense adjacency) Trainium2 Bass kernel.

Shards the batch (B=16) across 8 NeuronCores (2 samples/core), replicating
the small GAT weights. Per sample, on-device:
  h   = x @ W                      (PE, via PE-transposed x)
  e   = a_src/a_dst logit rows     (PE, small matmuls off h^T)
  p^T = mask * exp(prelu(e_dst[i] + e_src[j]))   (ACT Prelu+Exp, DVE mask)
  out = (p^T)^T-matmul h  / rowsum + bias        (PE agg with ones column)

Layouts: attention is built TRANSPOSED ([j, i], source nodes on partitions)
so softmax denominators and aggregation are both plain PE matmuls; adj is
PE-transposed on device; e_dst rows are broadcast across partitions via a
DRAM round-trip (partition-broadcast DMA reads).
"""

import numpy as np

import concourse.bass as bass
import concourse.bacc as bacc
import concourse.tile as tile
from concourse import mybir
from concourse.bass_utils import run_bass_kernel_spmd
from concourse.masks import make_identity

F32 = mybir.dt.float32
AF = mybir.ActivationFunctionType
ALU = mybir.AluOpType

P = 128          # partitions
N = 1024         # nodes
D = 256          # input feature dim
H = 4            # heads
F = 64           # per-head dim
HF = H * F       # 256
FA = F + 1       # head cols + ones column
NCH = N // P     # 8 chunks of nodes
NCORES = 8
BPC = 2          # batch samples per core
NEG_SLOPE = 0.2


def build_nc(num_devices=NCORES, repeat=1):
    nc = bacc.Bacc("TRN2", target_bir_lowering=False, debug=False,
                   num_devices=num_devices)
    x_d = nc.dram_tensor("x", [BPC, N, D], F32, kind="ExternalInput")
    adj_d = nc.dram_tensor("adj", [BPC, N, N], F32, kind="ExternalInput")
    w_d = nc.dram_tensor("W", [D, HF], F32, kind="ExternalInput")
    acat_d = nc.dram_tensor("acat", [HF, 2 * H], F32, kind="ExternalInput")
    bias_d = nc.dram_tensor("bias", [HF], F32, kind="ExternalInput")
    out_d = nc.dram_tensor("out", [BPC, N, HF], F32, kind="ExternalOutput")

    with tile.TileContext(nc) as tc:
        with (
            tc.tile_pool(name="consts", bufs=1) as consts,
            tc.tile_pool(name="xs", bufs=3) as p_xs,
            tc.tile_pool(name="xt", bufs=1) as p_xt,
            tc.tile_pool(name="haug", bufs=2) as p_haug,
            tc.tile_pool(name="ht", bufs=1) as p_ht,
            tc.tile_pool(name="erow", bufs=2) as p_erow,
            tc.tile_pool(name="ecol", bufs=2) as p_ecol,
            tc.tile_pool(name="bd", bufs=2) as p_bd,
            tc.tile_pool(name="mask", bufs=2) as p_mask,
            tc.tile_pool(name="adj", bufs=4) as p_adj,
            tc.tile_pool(name="pt", bufs=6) as p_pt,
            tc.tile_pool(name="ot", bufs=4) as p_ot,
            tc.tile_pool(name="ps", bufs=8, space="PSUM") as p_ps,
            tc.tile_pool(name="dram", bufs=2, space="DRAM") as p_dram,
        ):
            ident = consts.tile([P, P], F32)
            make_identity(nc, ident)
            w_sb = consts.tile([P, 2, HF], F32)
            acat_sb = consts.tile([P, 2, 2 * H], F32)
            for dc in range(2):
                nc.sync.dma_start(w_sb[:, dc, :], w_d[dc * P:(dc + 1) * P, :])
                nc.sync.dma_start(acat_sb[:, dc, :], acat_d[dc * P:(dc + 1) * P, :])
            bias_bc = consts.tile([P, HF], F32)
            nc.sync.dma_start(bias_bc[:], bias_d[:].partition_broadcast(P))
            alpha_col = consts.tile([P, 1], F32)
            nc.vector.memset(alpha_col[:], NEG_SLOPE)

            for _rep in range(repeat):
                for b in range(BPC):
                    # ---- Phase A: load x, PE-transpose to xT [d, i] ----
                    xt_t = p_xt.tile([P, 2, N], F32, tag="xt")
                    for icg in range(2):   # groups of 4 i-chunks
                        ps0 = p_ps.tile([P, 512], F32, tag="u")
                        ps1 = p_ps.tile([P, 512], F32, tag="u")
                        for ic4 in range(4):
                            ic = icg * 4 + ic4
                            xs = p_xs.tile([P, D], F32, tag="xs")
                            nc.sync.dma_start(xs[:], x_d[b, ic * P:(ic + 1) * P, :])
                            nc.tensor.transpose(ps0[:, ic4 * P:(ic4 + 1) * P],
                                                xs[:, 0:P], ident[:])
                            nc.tensor.transpose(ps1[:, ic4 * P:(ic4 + 1) * P],
                                                xs[:, P:D], ident[:])
                        nc.vector.tensor_copy(xt_t[:, 0, icg * 512:(icg + 1) * 512], ps0[:])
                        nc.vector.tensor_copy(xt_t[:, 1, icg * 512:(icg + 1) * 512], ps1[:])

                    # ---- Phase B: h_aug, hT, E rows, e columns ----
                    haug_t = p_haug.tile([P, NCH, H, FA], F32, tag="haug")
                    nc.gpsimd.memset(haug_t[:, :, :, F:FA], 1.0)
                    for ic in range(NCH):
                        ph = p_ps.tile([P, HF], F32, tag="u")
                        for dc in range(2):
                            nc.tensor.matmul(ph[:], xt_t[:, dc, ic * P:(ic + 1) * P],
                                             w_sb[:, dc, :],
                                             start=(dc == 0), stop=(dc == 1))
                        nc.vector.tensor_copy(
                            haug_t[:, ic, :, 0:F],
                            ph.rearrange("p (h f) -> p h f", h=H))

                    ht_t = p_ht.tile([P, 2, N], F32, tag="ht")
                    for dc2 in range(2):
                        for nh in range(2):
                            pht = p_ps.tile([P, 512], F32, tag="u")
                            for dc in range(2):
                                nc.tensor.matmul(
                                    pht[:],
                                    w_sb[:, dc, dc2 * P:(dc2 + 1) * P],
                                    xt_t[:, dc, nh * 512:(nh + 1) * 512],
                                    start=(dc == 0), stop=(dc == 1))
                            nc.vector.tensor_copy(ht_t[:, dc2, nh * 512:(nh + 1) * 512],
                                                  pht[:])

                    # E rows [2H, N]: row 2h = e_src head h, row 2h+1 = e_dst
                    erow_t = p_erow.tile([P, N], F32, tag="erow")
                    nc.gpsimd.memset(erow_t[:], 0.0)
                    for nh in range(2):
                        pe_ = p_ps.tile([P, 512], F32, tag="u")
                        for dc2 in range(2):
                            nc.tensor.matmul(pe_[0:2 * H, :],
                                             acat_sb[:, dc2, :],
                                             ht_t[:, dc2, nh * 512:(nh + 1) * 512],
                                             start=(dc2 == 0), stop=(dc2 == 1))
                        nc.vector.tensor_copy(erow_t[0:2 * H, nh * 512:(nh + 1) * 512],
                                              pe_[0:2 * H, :])

                    # e columns: transpose E rows -> [node_part, 2H] per chunk
                    ecol_t = p_ecol.tile([P, NCH, 2 * H], F32, tag="ecol")
                    for jc in range(NCH):
                        pec = p_ps.tile([P, P], F32, tag="u")
                        nc.tensor.transpose(pec[:], erow_t[:, jc * P:(jc + 1) * P],
                                            ident[:])
                        nc.vector.tensor_copy(ecol_t[:, jc, :], pec[:, 0:2 * H])

                    # e_dst rows broadcast across partitions via DRAM round-trip
                    scr = p_dram.tile([2 * H, N], F32, tag="scr")
                    nc.sync.dma_start(scr[:], erow_t[0:2 * H, :])
                    bd_t = p_bd.tile([P, H, N], F32, tag="bd")
                    for h in range(H):
                        nc.sync.dma_start(bd_t[:, h, :],
                                          scr[2 * h + 1, :].partition_broadcast(P))

                    # ---- Phase M: transposed edge mask (adj^T > 0.5) ----
                    mask_t = p_mask.tile([P, NCH, N], F32, tag="mask")
                    for jc in range(NCH):
                        pm0 = p_ps.tile([P, 512], F32, tag="u")
                        pm1 = p_ps.tile([P, 512], F32, tag="u")
                        for ib in range(NCH):
                            at = p_adj.tile([P, P], F32, tag="adj")
                            nc.sync.dma_start(
                                at[:], adj_d[b, ib * P:(ib + 1) * P,
                                             jc * P:(jc + 1) * P])
                            dst = pm0 if ib < 4 else pm1
                            nc.tensor.transpose(dst[:, (ib % 4) * P:(ib % 4 + 1) * P],
                                                at[:], ident[:])
                        nc.vector.tensor_scalar(out=mask_t[:, jc, 0:512], in0=pm0[:],
                                                scalar1=0.5, scalar2=None,
                                                op0=ALU.is_gt)
                        nc.vector.tensor_scalar(out=mask_t[:, jc, 512:N], in0=pm1[:],
                                                scalar1=0.5, scalar2=None,
                                                op0=ALU.is_gt)

                    # ---- Phase D: attention tiles + aggregation matmuls ----
                    pouts = [p_ps.tile([P, H * FA], F32, tag="u", name=f"pout{b}_{i}")
                             for i in range(NCH)]
                    for h in range(H):
                        for jc in range(NCH):
                            pt = p_pt.tile([P, N], F32, tag="pt")
                            # prelu(e_dst[i] + e_src[j]) ; j on partitions
                            nc.scalar.activation(
                                out=pt[:], in_=bd_t[:, h, :], func=AF.Prelu,
                                bias=ecol_t[:, jc, 2 * h:2 * h + 1],
                                scale=1.0, alpha=alpha_col[:])
                            nc.scalar.activation(out=pt[:], in_=pt[:], func=AF.Exp,
                                                 bias=0.0, scale=1.0)
                            nc.vector.tensor_tensor(out=pt[:], in0=pt[:],
                                                    in1=mask_t[:, jc, :],
                                                    op=ALU.mult)
                            first = (h == 0 and jc == 0)
                            last = (h == H - 1 and jc == NCH - 1)
                            for ic in range(NCH):
                                nc.tensor.matmul(
                                    pouts[ic][:, h * FA:(h + 1) * FA],
                                    pt[:, ic * P:(ic + 1) * P],
                                    haug_t[:, jc, h, :],
                                    start=first, stop=last)

                    # ---- Phase E: normalize + bias + store ----
                    for ic in range(NCH):
                        po = pouts[ic].rearrange("p (h f) -> p h f", h=H)
                        recip = p_ot.tile([P, H], F32, tag="recip")
                        nc.vector.reciprocal(recip[:], po[:, :, F])
                        ot = p_ot.tile([P, HF], F32, tag="ot")
                        otv = ot.rearrange("p (h f) -> p h f", h=H)
                        for h in range(H):
                            nc.vector.tensor_scalar(
                                out=otv[:, h, :], in0=po[:, h, 0:F],
                                scalar1=recip[:, h:h + 1], scalar2=None,
                                op0=ALU.mult)
                        nc.vector.tensor_tensor(out=ot[:], in0=ot[:], in1=bias_bc[:],
                                                op=ALU.add)
                        nc.sync.dma_start(out_d[b, ic * P:(ic + 1) * P, :], ot[:])

    nc.compile()
    return nc


_NC_CACHE = {}


def _get_nc():
    if "nc" not in _NC_CACHE:
        _NC_CACHE["nc"] = build_nc()
    return _NC_CACHE["nc"]


def _prep_weights(W, a_src, a_dst, bias):
    W2 = np.ascontiguousarray(W.reshape(D, HF).astype(np.float32))
    acat = np.zeros((HF, 2 * H), np.float32)
    for h in range(H):
        acat[h * F:(h + 1) * F, 2 * h] = a_src[h]
        acat[h * F:(h + 1) * F, 2 * h + 1] = a_dst[h]
    return W2, acat, np.ascontiguousarray(bias.astype(np.float32))


def kernel(x, adj, W, a_src, a_dst, bias):
    x = np.asarray(x, dtype=np.float32)
    adj = np.asarray(adj, dtype=np.float32)
    W2, acat, biasv = _prep_weights(np.asarray(W), np.asarray(a_src),
                                    np.asarray(a_dst), np.asarray(bias))
    nc = _get_nc()
    in_maps = []
    for c in range(NCORES):
        in_maps.append({
            "x": np.ascontiguousarray(x[c * BPC:(c + 1) * BPC]),
            "adj": np.ascontiguousarray(adj[c * BPC:(c + 1) * BPC]),
            "W": W2, "acat": acat, "bias": biasv,
        })
    r = run_bass_kernel_spmd(nc, in_maps, core_ids=list(range(NCORES)))
    return np.concatenate([r.results[c]["out"] for c in range(NCORES)], axis=0)


# revision 4
# speedup vs baseline: 424.9371x; 424.9371x over previous
"""Batched GAT (dense adjacency) Trainium2 Bass kernel.

Shards the batch (B=16) across 8 NeuronCores (2 samples/core), replicating
the small GAT weights. Per sample, on-device:
  h   = x @ W                      (PE, via PE-transposed x)
  e   = a_src/a_dst logit rows     (PE, small matmuls off h^T)
  p^T = mask * exp(prelu(e_dst[i] + e_src[j]))   (ACT Prelu+Exp, DVE mask)
  out = (p^T)^T-matmul h  / rowsum + bias        (PE agg with ones column)

Layouts: attention is built TRANSPOSED ([j, i], source nodes on partitions)
so softmax denominators and aggregation are both plain PE matmuls; adj is
PE-transposed on device; e_dst rows are broadcast across partitions via a
DRAM round-trip (partition-broadcast DMA reads).
"""

import numpy as np

import concourse.bass as bass
import concourse.bacc as bacc
import concourse.tile as tile
from concourse import mybir
from concourse.bass_utils import run_bass_kernel_spmd
from concourse.masks import make_identity

F32 = mybir.dt.float32
AF = mybir.ActivationFunctionType
ALU = mybir.AluOpType

P = 128          # partitions
N = 1024         # nodes
D = 256          # input feature dim
H = 4            # heads
F = 64           # per-head dim
HF = H * F       # 256
FA = F + 1       # head cols + ones column
NCH = N // P     # 8 chunks of nodes
NCORES = 8
BPC = 2          # batch samples per core
NEG_SLOPE = 0.2


def build_nc(num_devices=NCORES, repeat=1):
    nc = bacc.Bacc("TRN2", target_bir_lowering=False, debug=False,
                   num_devices=num_devices)
    x_d = nc.dram_tensor("x", [BPC, N, D], F32, kind="ExternalInput")
    adj_d = nc.dram_tensor("adj", [BPC, N, N], F32, kind="ExternalInput")
    w_d = nc.dram_tensor("W", [D, HF], F32, kind="ExternalInput")
    acat_d = nc.dram_tensor("acat", [HF, 2 * H], F32, kind="ExternalInput")
    bias_d = nc.dram_tensor("bias", [HF], F32, kind="ExternalInput")
    out_d = nc.dram_tensor("out", [BPC, N, HF], F32, kind="ExternalOutput")

    with tile.TileContext(nc) as tc:
        with (
            tc.tile_pool(name="consts", bufs=1) as consts,
            tc.tile_pool(name="xs", bufs=3) as p_xs,
            tc.tile_pool(name="xt", bufs=1) as p_xt,
            tc.tile_pool(name="haug", bufs=2) as p_haug,
            tc.tile_pool(name="ht", bufs=1) as p_ht,
            tc.tile_pool(name="erow", bufs=2) as p_erow,
            tc.tile_pool(name="ecol", bufs=2) as p_ecol,
            tc.tile_pool(name="bd", bufs=2) as p_bd,
            tc.tile_pool(name="mask", bufs=2) as p_mask,
            tc.tile_pool(name="adj", bufs=4) as p_adj,
            tc.tile_pool(name="pt", bufs=6) as p_pt,
            tc.tile_pool(name="ot", bufs=4) as p_ot,
            tc.tile_pool(name="ps", bufs=8, space="PSUM") as p_ps,
            tc.tile_pool(name="dram", bufs=2, space="DRAM") as p_dram,
        ):
            ident = consts.tile([P, P], F32)
            make_identity(nc, ident)
            w_sb = consts.tile([P, 2, HF], F32)
            acat_sb = consts.tile([P, 2, 2 * H], F32)
            for dc in range(2):
                nc.sync.dma_start(w_sb[:, dc, :], w_d[dc * P:(dc + 1) * P, :])
                nc.sync.dma_start(acat_sb[:, dc, :], acat_d[dc * P:(dc + 1) * P, :])
            bias_bc = consts.tile([P, HF], F32)
            nc.sync.dma_start(bias_bc[:], bias_d[:].partition_broadcast(P))
            alpha_col = consts.tile([P, 1], F32)
            nc.vector.memset(alpha_col[:], NEG_SLOPE)

            def body():
                for b in range(BPC):
                    # ---- Phase A: load x, PE-transpose to xT [d, i] ----
                    xt_t = p_xt.tile([P, 2, N], F32, tag="xt")
                    for icg in range(2):   # groups of 4 i-chunks
                        ps0 = p_ps.tile([P, 512], F32, tag="u")
                        ps1 = p_ps.tile([P, 512], F32, tag="u")
                        for ic4 in range(4):
                            ic = icg * 4 + ic4
                            xs = p_xs.tile([P, D], F32, tag="xs")
                            nc.sync.dma_start(xs[:], x_d[b, ic * P:(ic + 1) * P, :])
                            nc.tensor.transpose(ps0[:, ic4 * P:(ic4 + 1) * P],
                                                xs[:, 0:P], ident[:])
                            nc.tensor.transpose(ps1[:, ic4 * P:(ic4 + 1) * P],
                                                xs[:, P:D], ident[:])
                        nc.vector.tensor_copy(xt_t[:, 0, icg * 512:(icg + 1) * 512], ps0[:])
                        nc.vector.tensor_copy(xt_t[:, 1, icg * 512:(icg + 1) * 512], ps1[:])

                    # ---- Phase B: h_aug, hT, E rows, e columns ----
                    haug_t = p_haug.tile([P, NCH, H, FA], F32, tag="haug")
                    nc.gpsimd.memset(haug_t[:, :, :, F:FA], 1.0)
                    for ic in range(NCH):
                        ph = p_ps.tile([P, HF], F32, tag="u")
                        for dc in range(2):
                            nc.tensor.matmul(ph[:], xt_t[:, dc, ic * P:(ic + 1) * P],
                                             w_sb[:, dc, :],
                                             start=(dc == 0), stop=(dc == 1))
                        nc.vector.tensor_copy(
                            haug_t[:, ic, :, 0:F],
                            ph.rearrange("p (h f) -> p h f", h=H))

                    ht_t = p_ht.tile([P, 2, N], F32, tag="ht")
                    for dc2 in range(2):
                        for nh in range(2):
                            pht = p_ps.tile([P, 512], F32, tag="u")
                            for dc in range(2):
                                nc.tensor.matmul(
                                    pht[:],
                                    w_sb[:, dc, dc2 * P:(dc2 + 1) * P],
                                    xt_t[:, dc, nh * 512:(nh + 1) * 512],
                                    start=(dc == 0), stop=(dc == 1))
                            nc.vector.tensor_copy(ht_t[:, dc2, nh * 512:(nh + 1) * 512],
                                                  pht[:])

                    # E rows [2H, N]: row 2h = e_src head h, row 2h+1 = e_dst
                    erow_t = p_erow.tile([P, N], F32, tag="erow")
                    nc.gpsimd.memset(erow_t[:], 0.0)
                    for nh in range(2):
                        pe_ = p_ps.tile([P, 512], F32, tag="u")
                        for dc2 in range(2):
                            nc.tensor.matmul(pe_[0:2 * H, :],
                                             acat_sb[:, dc2, :],
                                             ht_t[:, dc2, nh * 512:(nh + 1) * 512],
                                             start=(dc2 == 0), stop=(dc2 == 1))
                        nc.vector.tensor_copy(erow_t[0:2 * H, nh * 512:(nh + 1) * 512],
                                              pe_[0:2 * H, :])

                    # e columns: transpose E rows -> [node_part, 2H] per chunk
                    ecol_t = p_ecol.tile([P, NCH, 2 * H], F32, tag="ecol")
                    for jc in range(NCH):
                        pec = p_ps.tile([P, P], F32, tag="u")
                        nc.tensor.transpose(pec[:], erow_t[:, jc * P:(jc + 1) * P],
                                            ident[:])
                        nc.vector.tensor_copy(ecol_t[:, jc, :], pec[:, 0:2 * H])

                    # e_dst rows broadcast across partitions via DRAM round-trip
                    scr = p_dram.tile([2 * H, N], F32, tag="scr")
                    nc.sync.dma_start(scr[:], erow_t[0:2 * H, :])
                    bd_t = p_bd.tile([P, H, N], F32, tag="bd")
                    for h in range(H):
                        nc.sync.dma_start(bd_t[:, h, :],
                                          scr[2 * h + 1, :].partition_broadcast(P))

                    # ---- Phase M: transposed edge mask (adj^T > 0.5) ----
                    mask_t = p_mask.tile([P, NCH, N], F32, tag="mask")
                    for jc in range(NCH):
                        pm0 = p_ps.tile([P, 512], F32, tag="u")
                        pm1 = p_ps.tile([P, 512], F32, tag="u")
                        for ib in range(NCH):
                            at = p_adj.tile([P, P], F32, tag="adj")
                            nc.sync.dma_start(
                                at[:], adj_d[b, ib * P:(ib + 1) * P,
                                             jc * P:(jc + 1) * P])
                            dst = pm0 if ib < 4 else pm1
                            nc.tensor.transpose(dst[:, (ib % 4) * P:(ib % 4 + 1) * P],
                                                at[:], ident[:])
                        nc.vector.tensor_scalar(out=mask_t[:, jc, 0:512], in0=pm0[:],
                                                scalar1=0.5, scalar2=None,
                                                op0=ALU.is_gt)
                        nc.vector.tensor_scalar(out=mask_t[:, jc, 512:N], in0=pm1[:],
                                                scalar1=0.5, scalar2=None,
                                                op0=ALU.is_gt)

                    # ---- Phase D: attention tiles + aggregation matmuls ----
                    pouts = [p_ps.tile([P, H * FA], F32, tag="u", name=f"pout{b}_{i}")
                             for i in range(NCH)]
                    for h in range(H):
                        for jc in range(NCH):
                            pt = p_pt.tile([P, N], F32, tag="pt")
                            # prelu(e_dst[i] + e_src[j]) ; j on partitions
                            nc.scalar.activation(
                                out=pt[:], in_=bd_t[:, h, :], func=AF.Prelu,
                                bias=ecol_t[:, jc, 2 * h:2 * h + 1],
                                scale=1.0, alpha=alpha_col[:])
                            nc.scalar.activation(out=pt[:], in_=pt[:], func=AF.Exp,
                                                 bias=0.0, scale=1.0)
                            nc.vector.tensor_tensor(out=pt[:], in0=pt[:],
                                                    in1=mask_t[:, jc, :],
                                                    op=ALU.mult)
                            first = (h == 0 and jc == 0)
                            last = (h == H - 1 and jc == NCH - 1)
                            for ic in range(NCH):
                                nc.tensor.matmul(
                                    pouts[ic][:, h * FA:(h + 1) * FA],
                                    pt[:, ic * P:(ic + 1) * P],
                                    haug_t[:, jc, h, :],
                                    start=first, stop=last)

                    # ---- Phase E: normalize + bias + store ----
                    for ic in range(NCH):
                        po = pouts[ic].rearrange("p (h f) -> p h f", h=H)
                        recip = p_ot.tile([P, H], F32, tag="recip")
                        nc.vector.reciprocal(recip[:], po[:, :, F])
                        ot = p_ot.tile([P, HF], F32, tag="ot")
                        otv = ot.rearrange("p (h f) -> p h f", h=H)
                        for h in range(H):
                            nc.vector.tensor_scalar(
                                out=otv[:, h, :], in0=po[:, h, 0:F],
                                scalar1=recip[:, h:h + 1], scalar2=None,
                                op0=ALU.mult)
                        nc.vector.tensor_tensor(out=ot[:], in0=ot[:], in1=bias_bc[:],
                                                op=ALU.add)
                        nc.sync.dma_start(out_d[b, ic * P:(ic + 1) * P, :], ot[:])

            if repeat == 1:
                body()
            else:
                with tc.For_i(0, repeat, 1):
                    body()

    nc.compile()
    return nc


_NC_CACHE = {}


def _get_nc():
    if "nc" not in _NC_CACHE:
        _NC_CACHE["nc"] = build_nc()
    return _NC_CACHE["nc"]


def _prep_weights(W, a_src, a_dst, bias):
    W2 = np.ascontiguousarray(W.reshape(D, HF).astype(np.float32))
    acat = np.zeros((HF, 2 * H), np.float32)
    for h in range(H):
        acat[h * F:(h + 1) * F, 2 * h] = a_src[h]
        acat[h * F:(h + 1) * F, 2 * h + 1] = a_dst[h]
    return W2, acat, np.ascontiguousarray(bias.astype(np.float32))


def kernel(x, adj, W, a_src, a_dst, bias):
    x = np.asarray(x, dtype=np.float32)
    adj = np.asarray(adj, dtype=np.float32)
    W2, acat, biasv = _prep_weights(np.asarray(W), np.asarray(a_src),
                                    np.asarray(a_dst), np.asarray(bias))
    nc = _get_nc()
    in_maps = []
    for c in range(NCORES):
        in_maps.append({
            "x": np.ascontiguousarray(x[c * BPC:(c + 1) * BPC]),
            "adj": np.ascontiguousarray(adj[c * BPC:(c + 1) * BPC]),
            "W": W2, "acat": acat, "bias": biasv,
        })
    r = run_bass_kernel_spmd(nc, in_maps, core_ids=list(range(NCORES)))
    return np.concatenate([r.results[c]["out"] for c in range(NCORES)], axis=0)


# revision 7
# speedup vs baseline: 792.8611x; 1.8658x over previous
"""Batched GAT (dense adjacency) Trainium2 Bass kernel.

Shards the batch (B=16) across 8 NeuronCores (2 samples/core), replicating
the small GAT weights. Per sample, on-device:
  h   = x @ W                      (PE, via PE-transposed x)
  e   = a_src/a_dst logit rows     (PE, small matmuls off h^T)
  p^T = mask * exp(prelu(e_dst[i] + e_src[j]))   (ACT Prelu+Exp, DVE mask)
  out = (p^T)^T-matmul h  / rowsum + bias        (PE agg with ones column)

Layouts: attention is built TRANSPOSED ([j, i], source nodes on partitions)
so softmax denominators and aggregation are both plain PE matmuls; adj is
PE-transposed on device; e_dst rows are broadcast across partitions via a
DRAM round-trip (partition-broadcast DMA reads). The aggregation runs in
bf16 (errors largely cancel between numerator and softmax denominator);
the logit path stays f32 with f32r (fast-fp32) used for the wide matmuls.
"""

import numpy as np

import concourse.bass as bass
import concourse.bacc as bacc
import concourse.tile as tile
from concourse import mybir
from concourse.bass_utils import run_bass_kernel_spmd
from concourse.masks import make_identity

F32 = mybir.dt.float32
F32R = mybir.dt.float32r
BF16 = mybir.dt.bfloat16
AF = mybir.ActivationFunctionType
ALU = mybir.AluOpType

P = 128          # partitions
N = 1024         # nodes
D = 256          # input feature dim
H = 4            # heads
F = 64           # per-head dim
HF = H * F       # 256
FA = F + 1       # head cols + ones column
NCH = N // P     # 8 chunks of nodes
NCORES = 8
BPC = 2          # batch samples per core
NEG_SLOPE = 0.2


def build_nc(num_devices=NCORES, repeat=1):
    nc = bacc.Bacc("TRN2", target_bir_lowering=False, debug=False,
                   num_devices=num_devices)
    x_d = nc.dram_tensor("x", [BPC, N, D], F32, kind="ExternalInput")
    adj_d = nc.dram_tensor("adj", [BPC, N, N], F32, kind="ExternalInput")
    w_d = nc.dram_tensor("W", [D, HF], F32, kind="ExternalInput")
    acat_d = nc.dram_tensor("acat", [HF, 2 * H], F32, kind="ExternalInput")
    bias_d = nc.dram_tensor("bias", [HF], F32, kind="ExternalInput")
    out_d = nc.dram_tensor("out", [BPC, N, HF], F32, kind="ExternalOutput")

    with tile.TileContext(nc) as tc:
        with (
            tc.tile_pool(name="consts", bufs=1) as consts,
            tc.tile_pool(name="xs", bufs=3) as p_xs,
            tc.tile_pool(name="xt", bufs=2) as p_xt,
            tc.tile_pool(name="haug", bufs=2) as p_haug,
            tc.tile_pool(name="ht", bufs=2) as p_ht,
            tc.tile_pool(name="erow", bufs=2) as p_erow,
            tc.tile_pool(name="ecol", bufs=2) as p_ecol,
            tc.tile_pool(name="bd", bufs=2) as p_bd,
            tc.tile_pool(name="mask", bufs=2) as p_mask,
            tc.tile_pool(name="adj", bufs=3) as p_adj,
            tc.tile_pool(name="pt", bufs=6) as p_pt,
            tc.tile_pool(name="pm", bufs=6) as p_pm,
            tc.tile_pool(name="ot", bufs=4) as p_ot,
            tc.tile_pool(name="ps", bufs=8, space="PSUM") as p_ps,
            tc.tile_pool(name="dram", bufs=2, space="DRAM") as p_dram,
        ):
            ident = consts.tile([P, P], F32)
            make_identity(nc, ident)
            w_sb = consts.tile([P, 2, HF], F32)
            acat_sb = consts.tile([P, 2, 2 * H], F32)
            for dc in range(2):
                nc.sync.dma_start(w_sb[:, dc, :], w_d[dc * P:(dc + 1) * P, :])
                nc.sync.dma_start(acat_sb[:, dc, :], acat_d[dc * P:(dc + 1) * P, :])
            bias_bc = consts.tile([P, HF], F32)
            nc.sync.dma_start(bias_bc[:], bias_d[:].partition_broadcast(P))
            w_sbr = consts.tile([P, 2, HF], F32R)
            nc.vector.tensor_copy(w_sbr[:], w_sb[:])
            acat_sbr = consts.tile([P, 2, 2 * H], F32R)
            nc.vector.tensor_copy(acat_sbr[:], acat_sb[:])
            alpha_col = consts.tile([P, 1], F32)
            nc.vector.memset(alpha_col[:], NEG_SLOPE)

            def phase_abm(b):
                """Load + transpose x, compute h_aug/hT/E/e-cols, mask."""
                # ---- A: load x, PE-transpose to xT [d, i] ----
                xt_t = p_xt.tile([P, 2, N], F32R, tag="xt", name=f"xt{b}")
                for icg in range(2):   # groups of 4 i-chunks
                    ps0 = p_ps.tile([P, 512], F32, tag="u", name=f"psx0_{b}{icg}")
                    ps1 = p_ps.tile([P, 512], F32, tag="u", name=f"psx1_{b}{icg}")
                    for ic4 in range(4):
                        ic = icg * 4 + ic4
                        xs = p_xs.tile([P, D], F32, tag="xs", name=f"xs{b}{ic}")
                        nc.sync.dma_start(xs[:], x_d[b, ic * P:(ic + 1) * P, :])
                        nc.tensor.transpose(ps0[:, ic4 * P:(ic4 + 1) * P],
                                            xs[:, 0:P], ident[:])
                        nc.tensor.transpose(ps1[:, ic4 * P:(ic4 + 1) * P],
                                            xs[:, P:D], ident[:])
                    nc.vector.tensor_copy(xt_t[:, 0, icg * 512:(icg + 1) * 512], ps0[:])
                    nc.vector.tensor_copy(xt_t[:, 1, icg * 512:(icg + 1) * 512], ps1[:])

                # ---- B: h_aug (bf16, ones col), hT, E rows, e cols ----
                haug_t = p_haug.tile([P, NCH, H, FA], BF16, tag="haug",
                                     name=f"haug{b}")
                nc.gpsimd.memset(haug_t[:, :, :, F:FA], 1.0)
                for ic in range(NCH):
                    ph = p_ps.tile([P, HF], F32, tag="u", name=f"psh{b}{ic}")
                    for dc in range(2):
                        nc.tensor.matmul(ph[:],
                                         xt_t[:, dc, ic * P:(ic + 1) * P],
                                         w_sbr[:, dc, :],
                                         start=(dc == 0), stop=(dc == 1))
                    nc.vector.tensor_copy(
                        haug_t[:, ic, :, 0:F],
                        ph.rearrange("p (h f) -> p h f", h=H))

                ht_t = p_ht.tile([P, 2, N], F32R, tag="ht", name=f"ht{b}")
                for dc2 in range(2):
                    for nh in range(2):
                        pht = p_ps.tile([P, 512], F32, tag="u",
                                        name=f"psht{b}{dc2}{nh}")
                        for dc in range(2):
                            nc.tensor.matmul(
                                pht[:],
                                w_sbr[:, dc, dc2 * P:(dc2 + 1) * P],
                                xt_t[:, dc, nh * 512:(nh + 1) * 512],
                                start=(dc == 0), stop=(dc == 1))
                        nc.vector.tensor_copy(ht_t[:, dc2, nh * 512:(nh + 1) * 512],
                                              pht[:])

                # E rows [2H, N]: row 2h = e_src head h, row 2h+1 = e_dst
                erow_t = p_erow.tile([P, N], F32, tag="erow", name=f"erow{b}")
                nc.gpsimd.memset(erow_t[:], 0.0)
                for nh in range(2):
                    pe_ = p_ps.tile([P, 512], F32, tag="u", name=f"pse{b}{nh}")
                    for dc2 in range(2):
                        nc.tensor.matmul(pe_[0:2 * H, :],
                                         acat_sbr[:, dc2, :],
                                         ht_t[:, dc2, nh * 512:(nh + 1) * 512],
                                         start=(dc2 == 0), stop=(dc2 == 1))
                    nc.vector.tensor_copy(erow_t[0:2 * H, nh * 512:(nh + 1) * 512],
                                          pe_[0:2 * H, :])

                # e columns: transpose E rows -> [node_part, 2H] per chunk
                ecol_t = p_ecol.tile([P, NCH, 2 * H], F32, tag="ecol",
                                     name=f"ecol{b}")
                for jc in range(NCH):
                    pec = p_ps.tile([P, P], F32, tag="u", name=f"pec{b}{jc}")
                    nc.tensor.transpose(pec[:], erow_t[:, jc * P:(jc + 1) * P],
                                        ident[:])
                    nc.vector.tensor_copy(ecol_t[:, jc, :], pec[:, 0:2 * H])

                # e_dst rows broadcast across partitions via DRAM round-trip
                scr = p_dram.tile([2 * H, N], F32, tag="scr", name=f"scr{b}")
                nc.sync.dma_start(scr[:], erow_t[0:2 * H, :])
                bd_t = p_bd.tile([P, H, N], F32, tag="bd", name=f"bd{b}")
                for h in range(H):
                    nc.sync.dma_start(bd_t[:, h, :],
                                      scr[2 * h + 1, :].partition_broadcast(P))

                # ---- M: transposed edge mask (adj^T > 0.5), bf16 ----
                mask_t = p_mask.tile([P, NCH, N], BF16, tag="mask",
                                     name=f"mask{b}")
                for jcp in range(4):   # pairs of j-chunks
                    pms = [p_ps.tile([P, 512], F32, tag="u",
                                     name=f"pm{b}{jcp}{q}") for q in range(4)]
                    for ib in range(NCH):
                        at = p_adj.tile([P, 2 * P], F32, tag="adj",
                                        name=f"at{b}{jcp}{ib}")
                        nc.sync.dma_start(
                            at[:], adj_d[b, ib * P:(ib + 1) * P,
                                         jcp * 2 * P:(jcp + 1) * 2 * P])
                        q = ib // 4
                        nc.tensor.transpose(pms[q][:, (ib % 4) * P:(ib % 4 + 1) * P],
                                            at[:, 0:P], ident[:])
                        nc.tensor.transpose(pms[2 + q][:, (ib % 4) * P:(ib % 4 + 1) * P],
                                            at[:, P:2 * P], ident[:])
                    for q in range(4):
                        jc = jcp * 2 + q // 2
                        half = (q % 2) * 512
                        nc.vector.tensor_scalar(
                            out=mask_t[:, jc, half:half + 512],
                            in0=pms[q][:],
                            scalar1=0.5, scalar2=None, op0=ALU.is_gt)
                return haug_t, ecol_t, bd_t, mask_t

            def phase_de(b, haug_t, ecol_t, bd_t, mask_t):
                # ---- D: attention tiles + aggregation matmuls ----
                pouts = [p_ps.tile([P, H * FA], F32, tag="u", name=f"po{b}_{i}")
                         for i in range(NCH)]
                for h in range(H):
                    for jc in range(NCH):
                        pt = p_pt.tile([P, N], F32, tag="pt", name=f"pt{b}{h}{jc}")
                        nc.scalar.activation(
                            out=pt[:], in_=bd_t[:, h, :], func=AF.Prelu,
                            bias=ecol_t[:, jc, 2 * h:2 * h + 1],
                            scale=1.0, alpha=alpha_col[:])
                        nc.scalar.activation(out=pt[:], in_=pt[:], func=AF.Exp,
                                             bias=0.0, scale=1.0)
                        pm = p_pm.tile([P, N], BF16, tag="pm", name=f"pm{b}{h}{jc}")
                        nc.vector.tensor_tensor(out=pm[:], in0=pt[:],
                                                in1=mask_t[:, jc, :],
                                                op=ALU.mult)
                        first = (h == 0 and jc == 0)
                        last = (h == H - 1 and jc == NCH - 1)
                        for ic in range(NCH):
                            nc.tensor.matmul(
                                pouts[ic][:, h * FA:(h + 1) * FA],
                                pm[:, ic * P:(ic + 1) * P],
                                haug_t[:, jc, h, :],
                                start=first, stop=last)

                # ---- E: normalize + bias + store ----
                for ic in range(NCH):
                    po = pouts[ic].rearrange("p (h f) -> p h f", h=H)
                    recip = p_ot.tile([P, H], F32, tag="recip", name=f"rc{b}{ic}")
                    nc.vector.reciprocal(recip[:], po[:, :, F])
                    ot = p_ot.tile([P, HF], F32, tag="ot", name=f"ot{b}{ic}")
                    otv = ot.rearrange("p (h f) -> p h f", h=H)
                    for h in range(H):
                        nc.vector.tensor_scalar(
                            out=otv[:, h, :], in0=po[:, h, 0:F],
                            scalar1=recip[:, h:h + 1], scalar2=None,
                            op0=ALU.mult)
                    nc.gpsimd.tensor_tensor(out=ot[:], in0=ot[:], in1=bias_bc[:],
                                            op=ALU.add)
                    nc.sync.dma_start(out_d[b, ic * P:(ic + 1) * P, :], ot[:])

            def body():
                st = [phase_abm(b) for b in range(BPC)]
                for b in range(BPC):
                    phase_de(b, *st[b])

            if repeat == 1:
                body()
            else:
                with tc.For_i(0, repeat, 1):
                    body()

    nc.compile()
    return nc


_NC_CACHE = {}


def _get_nc():
    if "nc" not in _NC_CACHE:
        _NC_CACHE["nc"] = build_nc()
    return _NC_CACHE["nc"]


def _prep_weights(W, a_src, a_dst, bias):
    W2 = np.ascontiguousarray(W.reshape(D, HF).astype(np.float32))
    acat = np.zeros((HF, 2 * H), np.float32)
    for h in range(H):
        acat[h * F:(h + 1) * F, 2 * h] = a_src[h]
        acat[h * F:(h + 1) * F, 2 * h + 1] = a_dst[h]
    return W2, acat, np.ascontiguousarray(bias.astype(np.float32))


def kernel(x, adj, W, a_src, a_dst, bias):
    x = np.asarray(x, dtype=np.float32)
    adj = np.asarray(adj, dtype=np.float32)
    W2, acat, biasv = _prep_weights(np.asarray(W), np.asarray(a_src),
                                    np.asarray(a_dst), np.asarray(bias))
    nc = _get_nc()
    in_maps = []
    for c in range(NCORES):
        in_maps.append({
            "x": np.ascontiguousarray(x[c * BPC:(c + 1) * BPC]),
            "adj": np.ascontiguousarray(adj[c * BPC:(c + 1) * BPC]),
            "W": W2, "acat": acat, "bias": biasv,
        })
    r = run_bass_kernel_spmd(nc, in_maps, core_ids=list(range(NCORES)))
    return np.concatenate([r.results[c]["out"] for c in range(NCORES)], axis=0)
